# revision 1
# baseline (speedup 1.0000x reference)
"""Causal self-attention (B=4, T=2048, C=768, H=12) on 8 Trainium2 cores.

Sharding: core c handles batch b=c//2 and heads [6*(c%2), 6*(c%2)+6).
Each core computes its 6 heads end-to-end (qkv proj -> attention -> partial
c_proj); the host sums the two partial c_proj outputs per batch and adds
b_proj (tensor-parallel all-reduce done host-side).

Matmuls run in float32r (fp32 storage, reduced-precision multiplies at 4x
the fp32 PE rate); accumulation stays fp32 in PSUM.

Attention per head with S computed transposed (keys on partitions):
  S.T[k,q] = K.T @ Q per 128-key block; expS = exp(S.T * 1/sqrt(D)) fused on
  ScalarE (no max subtraction -- scores are bounded, exp stays in fp32 range);
  causal mask via affine_select on the diagonal block only;
  O'[65,q] += V'[kblock].T @ expS in PSUM, where V' carries an all-ones
  column so O'[64] accumulates the softmax denominators for free;
  O = O'[0:64] * recip(O'[64]) with a stream_shuffle partition broadcast.

Emission interleaves q/k projection with attention heads so ScalarE exp work
overlaps projection PE work (PSUM split: st 4 banks, O' 2, projections 2).
"""

import sys

sys.path.insert(0, "/opt/trn_rl_repo")

from contextlib import ExitStack

import numpy as np

import concourse.bass as bass
import concourse.tile as tile
from concourse import bacc, mybir, bass_utils

B, T, C, H = 4, 2048, 768, 12
D = C // H  # 64
HPC = H // 2  # heads per core = 6
NCORES = 8
QKC = 2 * HPC * D  # 768 q+k outcols per core
VC = HPC * (D + 1)  # 390 v cols (64 v + 1 ones per head)
KB = T // 128  # 16 key blocks
TB = T // 128  # 16 token blocks
CB = C // 128  # 6 contraction chunks
HT = T // 2  # 1024, q-half width

f32 = mybir.dt.float32
f32r = mybir.dt.float32r
ts = bass.ts
SCALE = 1.0 / float(np.sqrt(D))


def _emit(tc, xT, wqk, bqk, wv, wpc, y, dbg=None):
    nc = tc.nc
    Exp = mybir.ActivationFunctionType.Exp

    with ExitStack() as top:
        qkTp = top.enter_context(tc.tile_pool(name="qkTp", bufs=1))
        vtp = top.enter_context(tc.tile_pool(name="vtp", bufs=1))
        ocp = top.enter_context(tc.tile_pool(name="ocp", bufs=1))
        wp = top.enter_context(tc.tile_pool(name="wp", bufs=1))
        esp = top.enter_context(tc.tile_pool(name="esp", bufs=4))
        nrm = top.enter_context(tc.tile_pool(name="nrm", bufs=1))
        ohp = top.enter_context(tc.tile_pool(name="ohp", bufs=2))

        qkt = [qkTp.tile([128, T], f32r, tag=f"qkt{i}", name=f"qkt{i}") for i in range(CB)]
        vt = [vtp.tile([128, VC], f32r, tag=f"vt{t}", name=f"vt{t}") for t in range(TB)]
        ocat = [ocp.tile([128, T], f32r, tag=f"oc{i}", name=f"oc{i}") for i in range(3)]

        wqkt = [wp.tile([128, QKC], f32r, tag=f"wqk{i}", name=f"wqk{i}") for i in range(CB)]
        bqt = [wp.tile([128, 1], f32, tag=f"bq{i}", name=f"bq{i}") for i in range(CB)]

        def emit_qk_proj(ob, qkps):
            # qkT[128*ob : 128*(ob+1), :] = W@x.T + bias, token chunks of 512
            for tch in range(4):
                ps = qkps.tile([128, 512], f32, tag="qkps", name="qkps")
                for kc in range(CB):
                    nc.tensor.matmul(
                        ps[:],
                        wqkt[kc][:, ts(ob, 128)],
                        xt[kc][:, ts(tch, 512)],
                        start=(kc == 0),
                        stop=(kc == CB - 1),
                    )
                nc.vector.tensor_scalar_add(
                    qkt[ob][:, ts(tch, 512)], ps[:], bqt[ob][:, 0:1]
                )

        def norm_half(h, op, half):
            # O = O'[0:64] / O'[64] for q-half `half`; write into ocat
            bp = 64 * (h % 2)
            rb = nrm.tile([96, HT], f32, tag="rb", name="rb")
            nc.vector.tensor_copy(rb[D : D + 1, :], op[D : D + 1, :])
            nc.vector.stream_shuffle(rb[32:64, :], rb[64:96, :], mask=[0] * 32)
            nc.gpsimd.tensor_copy(rb[0:32, :], rb[32:64, :])
            rb2 = nrm.tile([64, HT], f32, tag="rb2", name="rb2")
            nc.vector.reciprocal_approx_fast(out=rb2[:], in_=rb[0:D, :])
            oh = ohp.tile([64, HT], f32r, tag="oh", name="oh")
            nc.vector.tensor_mul(oh[:], op[0:D, :], rb2[:])
            nc.sync.dma_start(
                ocat[h // 2][bp : bp + 64, half * HT : half * HT + HT], oh[:]
            )

        def emit_head(h, stp, opp):
            qt = qkt[h // 2]
            kt = qkt[3 + h // 2]
            bp = 64 * (h % 2)
            for half in range(2):
                hlo = half * HT
                hhi = hlo + HT
                op = opp.tile([D + 1, HT], f32, tag="op", name="op")
                for kb in range(min(KB, (hhi + 127) // 128)):
                    qs = 128 * kb
                    lo = max(hlo, qs)
                    if lo >= hhi:
                        continue
                    st = stp.tile([128, HT], f32, tag="st", name="st")
                    es = esp.tile([128, HT], f32r, tag="es", name="es")
                    # QK: S.T[k, q] in pieces of <=512 within one PSUM bank
                    a = lo
                    while a < hhi:
                        b = min((a // 512 + 1) * 512, hhi)
                        nc.tensor.matmul(
                            st[:, a - hlo : b - hlo],
                            kt[bp : bp + 64, ts(kb, 128)],
                            qt[bp : bp + 64, a:b],
                            start=True,
                            stop=True,
                        )
                        a = b
                    nc.scalar.activation(
                        es[:, lo - hlo : HT], st[:, lo - hlo : HT], Exp, scale=SCALE
                    )
                    diag = hlo <= qs
                    if diag:
                        # causal mask on the diagonal block: keep q >= k
                        nc.gpsimd.affine_select(
                            out=es[:, qs - hlo : qs - hlo + 128],
                            in_=es[:, qs - hlo : qs - hlo + 128],
                            compare_op=mybir.AluOpType.is_ge,
                            fill=0.0,
                            base=0,
                            pattern=[[1, 128]],
                            channel_multiplier=-1,
                        )
                    # PV accumulate into O' (q-half slice); the diagonal
                    # 128-wide piece is split out so the rest doesn't wait
                    # on the mask
                    pieces = []
                    for qc in range(max(kb // 4, 2 * half), 2 * half + 2):
                        a = max(qs, 512 * qc)
                        b = 512 * (qc + 1)
                        if diag and a == qs and b > qs + 256:
                            # split so the non-diagonal part doesn't wait on
                            # the mask; start=True only on the first piece --
                            # it clears the whole bank's has_written bits
                            pieces.append((qs, qs + 256, kb == 0, False))
                            pieces.append((qs + 256, b, False, kb == 4 * qc + 3))
                        else:
                            pieces.append((a, b, kb == 0, kb == 4 * qc + 3))
                    for a, b, start_f, stop_f in pieces:
                        nc.tensor.matmul(
                            op[:, a - hlo : b - hlo],
                            vt[kb][:, 65 * h : 65 * h + 65],
                            es[:, a - hlo : b - hlo],
                            start=start_f,
                            stop=stop_f,
                        )
                norm_half(h, op, half)

        # ---------------- emission: projections interleaved with heads ----
        with ExitStack() as psA:
            stp = psA.enter_context(tc.tile_pool(name="stp", bufs=2, space="PSUM"))
            op1 = psA.enter_context(ExitStack())
            opp = op1.enter_context(tc.tile_pool(name="opp", bufs=1, space="PSUM"))

            with ExitStack() as xsc:
                xw = xsc.enter_context(tc.tile_pool(name="xw", bufs=1))
                xt = [
                    xw.tile([128, T], f32r, tag=f"xt{i}", name=f"xt{i}")
                    for i in range(CB)
                ]
                with ExitStack() as wvsc:
                    wvp = wvsc.enter_context(tc.tile_pool(name="wvp", bufs=1))
                    wvt = [
                        wvp.tile([128, VC], f32r, tag=f"wv{i}", name=f"wv{i}")
                        for i in range(CB)
                    ]
                    ones128 = wvp.tile([1, 128], f32r, tag="ones128", name="ones128")
                    nc.sync.dma_start(ones128[:], xT[C : C + 1, 0:128])
                    wvb = wvp.tile([1, VC], f32r, tag="wvb", name="wvb")
                    nc.sync.dma_start(wvb[:], wv[C : C + 1, :])
                    for i in range(CB):
                        nc.sync.dma_start(wvt[i][:], wv[ts(i, 128), :])
                        nc.sync.dma_start(bqt[i][:], bqk[ts(i, 128), :])
                    for tch in range(4):
                        for i in range(CB):
                            nc.sync.dma_start(
                                xt[i][:, ts(tch, 512)], xT[ts(i, 128), ts(tch, 512)]
                            )
                        if tch < 2:
                            for i in range(3 * tch, 3 * tch + 3):
                                nc.sync.dma_start(wqkt[i][:], wqk[ts(i, 128), :])

                    with tc.tile_pool(name="vps", bufs=2, space="PSUM") as vps:
                        for tb in range(TB):
                            ps = vps.tile([128, VC], f32, tag="vps", name="vps")
                            for kc in range(CB + 1):
                                if kc < CB:
                                    lhsT = xt[kc][:, ts(tb, 128)]
                                    rhs = wvt[kc][:]
                                else:
                                    lhsT = ones128[:, 0:128]
                                    rhs = wvb[:]
                                nc.tensor.matmul(
                                    ps[:], lhsT, rhs, start=(kc == 0), stop=(kc == CB)
                                )
                            nc.vector.tensor_copy(vt[tb][:], ps[:])

                with tc.tile_pool(name="qkps", bufs=2, space="PSUM") as qkps:
                    emit_qk_proj(0, qkps)
                    emit_qk_proj(3, qkps)
                    emit_head(0, stp, opp)
                    emit_qk_proj(1, qkps)
                    emit_qk_proj(4, qkps)
                    emit_head(1, stp, opp)
                    emit_qk_proj(2, qkps)
                    emit_qk_proj(5, qkps)

            op1.close()
            with tc.tile_pool(name="opp2", bufs=2, space="PSUM") as opp2:
                emit_head(2, stp, opp2)
                emit_head(3, stp, opp2)
                emit_head(4, stp, opp2)
                emit_head(5, stp, opp2)

        if dbg is not None:
            for i in range(CB):
                nc.sync.dma_start(dbg["qkT"][ts(i, 128), :], qkt[i][:].bitcast(f32))
            for t in range(TB):
                nc.sync.dma_start(dbg["v"][ts(t, 128), :], vt[t][:].bitcast(f32))
            for i in range(3):
                nc.sync.dma_start(dbg["oc"][ts(i, 128), :], ocat[i][:].bitcast(f32))

        # ---------------- output projection ----------------
        with ExitStack() as phC:
            wpp = phC.enter_context(tc.tile_pool(name="wpp", bufs=1))
            yop = phC.enter_context(tc.tile_pool(name="yop", bufs=3))
            yps = phC.enter_context(tc.tile_pool(name="yps", bufs=2, space="PSUM"))

            wpt = [wpp.tile([128, C], f32r, tag=f"wp{i}", name=f"wp{i}") for i in range(3)]
            for i in range(3):
                nc.sync.dma_start(wpt[i][:], wpc[ts(i, 128), :])

            for tb in range(TB):
                ps = yps.tile([128, C], f32, tag="yps", name="yps")
                for kc in range(3):
                    for a, w in ((0, 512), (512, 256)):
                        nc.tensor.matmul(
                            ps[:, a : a + w],
                            ocat[kc][:, ts(tb, 128)],
                            wpt[kc][:, a : a + w],
                            start=(kc == 0),
                            stop=(kc == 2),
                        )
                yt = yop.tile([128, C], f32, tag="yt", name="yt")
                if tb % 2 == 0:
                    nc.vector.tensor_copy(yt[:], ps[:])
                else:
                    nc.scalar.activation(
                        yt[:], ps[:], mybir.ActivationFunctionType.Copy
                    )
                nc.sync.dma_start(y[ts(tb, 128), :], yt[:])


_PROGRAM = None


def _build():
    global _PROGRAM
    if _PROGRAM is not None:
        return _PROGRAM
    nc = bacc.Bacc("TRN2", target_bir_lowering=False, debug=False, num_devices=NCORES)
    xT = nc.dram_tensor("xT", [C + 1, T], f32r, kind="ExternalInput").ap()
    wqk = nc.dram_tensor("wqk", [C, QKC], f32r, kind="ExternalInput").ap()
    bqk = nc.dram_tensor("bqk", [QKC, 1], f32, kind="ExternalInput").ap()
    wv = nc.dram_tensor("wv", [C + 1, VC], f32r, kind="ExternalInput").ap()
    wpc = nc.dram_tensor("wpc", [HPC * D, C], f32r, kind="ExternalInput").ap()
    y = nc.dram_tensor("y", [T, C], f32, kind="ExternalOutput").ap()
    with tile.TileContext(nc) as tc:
        _emit(tc, xT, wqk, bqk, wv, wpc, y)
    nc.compile()
    _PROGRAM = nc
    return nc


def _in_maps(x, w_qkv, b_qkv, w_proj):
    maps = []
    for c in range(NCORES):
        b = c // 2
        half = c % 2
        h0 = HPC * half  # first global head
        r0 = D * h0  # row offset within each of q/k/v sections
        span = HPC * D  # 384

        xTb = np.vstack([x[b].T, np.ones((1, T), np.float32)])  # [C+1, T]

        wq = w_qkv[r0 : r0 + span, :]
        wk = w_qkv[C + r0 : C + r0 + span, :]
        wqk = np.ascontiguousarray(np.vstack([wq, wk]).T)  # [C, 768]
        bqk = np.concatenate(
            [b_qkv[r0 : r0 + span], b_qkv[C + r0 : C + r0 + span]]
        ).reshape(QKC, 1)

        wv = np.zeros((C + 1, VC), dtype=np.float32)
        for hl in range(HPC):
            g = 2 * C + r0 + D * hl
            wv[0:C, 65 * hl : 65 * hl + D] = w_qkv[g : g + D, :].T
            wv[C, 65 * hl : 65 * hl + D] = b_qkv[g : g + D]
            wv[C, 65 * hl + D] = 1.0

        wpc = np.ascontiguousarray(w_proj[:, r0 : r0 + span].T)  # [384, C]

        maps.append(
            {
                "xT": xTb.astype(np.float32),
                "wqk": wqk.astype(np.float32),
                "bqk": bqk.astype(np.float32),
                "wv": wv,
                "wpc": wpc.astype(np.float32),
            }
        )
    return maps


def kernel(x, w_qkv, b_qkv, w_proj, b_proj, _trace=False):
    x = np.asarray(x, dtype=np.float32)
    w_qkv = np.asarray(w_qkv, dtype=np.float32)
    b_qkv = np.asarray(b_qkv, dtype=np.float32)
    w_proj = np.asarray(w_proj, dtype=np.float32)
    b_proj = np.asarray(b_proj, dtype=np.float32)

    nc = _build()
    maps = _in_maps(x, w_qkv, b_qkv, w_proj)
    res = bass_utils.run_bass_kernel_spmd(
        nc, maps, core_ids=list(range(NCORES)), trace=_trace
    )
    out = np.empty((B, T, C), dtype=np.float32)
    for b in range(B):
        out[b] = res.results[2 * b]["y"] + res.results[2 * b + 1]["y"] + b_proj
    if _trace:
        kernel._last_exec_time_ns = res.exec_time_ns
        kernel._last_results = res
    return out



# revision 24
# speedup vs baseline: 1.2905x; 1.2905x over previous
"""Causal self-attention (B=4, T=2048, C=768, H=12) on 8 Trainium2 cores.

Sharding: core c handles batch b=c//2 and heads [6*(c%2), 6*(c%2)+6).
Each core computes its 6 heads end-to-end (qkv proj -> attention -> partial
c_proj); the host sums the two partial c_proj outputs per batch and adds
b_proj plus the (constant) v-bias contribution Wp @ b_v.

Precision/engine strategy (cost model: matmul = out_rows * cycles_per_row,
fp8e4+DoubleRow = 0.5 cyc/row and contracts 2x128 per instruction):
  - qkv projection: fp8 DoubleRow over C=768 (3 instructions per tile).
    Weights are pre-scaled x64 on the host (fp8e4 subnormal floor), the
    PSUM->SBUF copy rescales by 1/64 and adds the q/k bias per partition.
  - QK^T: fp8 DoubleRow with the head dim split as [32 partitions x 2
    column blocks]; q/k live in fp8 tiles laid out [32p, 2, T] per head.
  - softmax exp: no max-subtraction (scores are bounded). Split across
    ScalarE (exact exp) and DVE/Pool using the Schraudolph bit trick:
    f16_bits(e^(s*scale)) ~= int16(s * 23.08 + B), saturating.
  - PV: out[q, 65] form (es as stationary), f16, accumulated across key
    blocks in PSUM; V' carries a ones column so col 64 collects the
    softmax denominator for free. Normalization is a per-partition
    reciprocal + scalar multiply.
  - O^T for c_proj via batched DMA XBAR transposes (f16), c_proj in f16,
    y output in f16.
"""

import sys

sys.path.insert(0, "/opt/trn_rl_repo")

from contextlib import ExitStack

import numpy as np
import ml_dtypes

import concourse.bass as bass
import concourse.tile as tile
from concourse import bacc, mybir, bass_utils

B, T, C, H = 4, 2048, 768, 12
D = C // H  # 64
HPC = H // 2  # heads per core = 6
NCORES = 8
CB = C // 128  # 6 contraction chunks
VC = HPC * (D + 1)  # 390 v cols (64 v + 1 ones per head)
KB = T // 128  # 16 key blocks
HT = T // 2  # 1024 = q-half width

f32 = mybir.dt.float32
f16 = mybir.dt.float16
f8 = mybir.dt.float8e4
i16 = mybir.dt.int16
F8NP = mybir.dt.np(f8)
F16NP = mybir.dt.np(f16)

SCALE = 1.0 / float(np.sqrt(D))
WSCALE = 64.0  # fp8 weight pre-scale (host) / PSUM rescale (device)
Exp = mybir.ActivationFunctionType.Exp
Copy = mybir.ActivationFunctionType.Copy
Ident = mybir.ActivationFunctionType.Identity
DR = mybir.MatmulPerfMode.DoubleRow
MULT = mybir.AluOpType.mult
ADD = mybir.AluOpType.add

# Schraudolph exp in f16 bit space: bits(e^(x*SCALE)) ~ x*EXP_A + EXP_B
EXP_A = 1024.0 / float(np.log(2.0)) * SCALE
EXP_B = 15315.5  # (15<<10) + mid-error correction, tuned numerically

# exp engine balance: ScalarE (exact, 0.833ns/col) vs DVE (Schraudolph,
# 1.04ns/col). DVE starts pre-loaded to account for its norm/recip duties.
EXP_W_ACT = 1.2
EXP_W_DVE = 0.96
EXP_DVE_PRIME = 15000.0


def _exp_engine_picker():
    load = {"act": 0.0, "dve": EXP_DVE_PRIME}
    w = {"act": EXP_W_ACT, "dve": EXP_W_DVE}

    def pick(span):
        e = min(load, key=lambda k: (load[k] + span) / w[k])
        load[e] += span
        return e

    return pick


def _emit(tc, x8d, w8d, bqd, xfd, wvd, wpd, y, dbg=None):
    nc = tc.nc

    with ExitStack() as top:
        pers = top.enter_context(tc.tile_pool(name="pers", bufs=1))
        esp = top.enter_context(tc.tile_pool(name="esp", bufs=4))
        nrm = top.enter_context(tc.tile_pool(name="nrm", bufs=2))
        opre_p = top.enter_context(tc.tile_pool(name="opre_p", bufs=2))
        yop = top.enter_context(tc.tile_pool(name="yop", bufs=3))

        x8t = pers.tile([128, CB, T], f8, tag="x8t", name="x8t")
        w8t = pers.tile([128, CB, 768], f8, tag="w8t", name="w8t")
        xft = pers.tile([128, CB, T], f16, tag="xft", name="xft")
        wvt = pers.tile([128, CB, VC], f16, tag="wvt", name="wvt")
        bqt = pers.tile([128, 6], f32, tag="bqt", name="bqt")
        # q8a/k8a: heads 0,1,2 at partition bases 0/32/64 (d split [32p x 2]).
        # q8b/k8b: h4 at base 0, h5 at base 32, h3 at base 64.
        q8a = pers.tile([128, 2, T], f8, tag="q8a", name="q8a")
        k8a = pers.tile([128, 2, T], f8, tag="k8a", name="k8a")
        q8b = pers.tile([128, 2, T], f8, tag="q8b", name="q8b")
        k8b = pers.tile([128, 2, T], f8, tag="k8b", name="k8b")
        stageq = pers.tile([128, 3 * T], f8, tag="stageq", name="stageq")
        stagek = pers.tile([128, 3 * T], f8, tag="stagek", name="stagek")
        vt = [pers.tile([128, VC], f16, tag=f"vt{t}", name=f"vt{t}") for t in range(KB)]
        ocat = [
            pers.tile([128, T], f16, tag=f"oc{i}", name=f"oc{i}") for i in range(3)
        ]
        wpt = pers.tile([128, 3, 768], f16, tag="wpt", name="wpt")

        # ---------------- input DMAs (ordered for early compute start) ----
        for p in range(3):
            nc.sync.dma_start(w8t[:, 2 * p : 2 * p + 2, :], w8d[:, 2 * p : 2 * p + 2, :])
        for kc in range(CB):
            nc.sync.dma_start(x8t[:, kc, 0:1024], x8d[:, kc, 0:1024])
        nc.sync.dma_start(bqt[:], bqd[:])
        for kc in range(CB):
            nc.sync.dma_start(x8t[:, kc, 1024:T], x8d[:, kc, 1024:T])
        for kc in range(CB):
            nc.sync.dma_start(xft[:, kc, 0:1024], xfd[:, kc, 0:1024])
        nc.sync.dma_start(wvt[:], wvd[:])
        for kc in range(CB):
            nc.sync.dma_start(xft[:, kc, 1024:T], xfd[:, kc, 1024:T])
        nc.sync.dma_start(wpt[:], wpd[:])

        # ---------------- qkv projection (fp8 DoubleRow) ----------------
        # proj block ob holds 128 q-or-k features in DR layout column order.
        with tc.tile_pool(name="qkps", bufs=2, space="PSUM") as qkps:
            for tch in range(4):
                c0, c1 = 512 * tch, 512 * (tch + 1)
                for ob in range(6):
                    ps = qkps.tile([128, 512], f32, tag="qkps", name="qkps")
                    for p in range(3):
                        nc.tensor.matmul(
                            ps[:],
                            w8t[:, 2 * p : 2 * p + 2, 128 * ob : 128 * (ob + 1)],
                            x8t[:, 2 * p : 2 * p + 2, c0:c1],
                            start=(p == 0),
                            stop=(p == 2),
                            perf_mode=DR,
                        )
                    bias = bqt[:, ob : ob + 1]
                    t8a, t8b, stg = (q8a, q8b, stageq) if ob < 3 else (k8a, k8b, stagek)
                    j = ob % 3
                    if j < 2:
                        # heads 0-2 direct; h3 (parts 96:128) staged for DMA
                        nc.vector.tensor_scalar(
                            t8a[0:96, j, c0:c1], ps[0:96, :], 1.0 / WSCALE, bias[0:96],
                            op0=MULT, op1=ADD,
                        )
                        nc.vector.tensor_scalar(
                            stg[96:128, T * j + c0 : T * j + c1], ps[96:128, :],
                            1.0 / WSCALE, bias[96:128], op0=MULT, op1=ADD,
                        )
                    else:
                        # h4/h5 d-lo direct; d-hi (parts 64:128) staged
                        nc.vector.tensor_scalar(
                            t8b[0:64, 0, c0:c1], ps[0:64, :], 1.0 / WSCALE, bias[0:64],
                            op0=MULT, op1=ADD,
                        )
                        nc.vector.tensor_scalar(
                            stg[64:128, 2 * T + c0 : 2 * T + c1], ps[64:128, :],
                            1.0 / WSCALE, bias[64:128], op0=MULT, op1=ADD,
                        )
            for t8b, stg in ((q8b, stageq), (k8b, stagek)):
                nc.sync.dma_start(t8b[64:96, 0, :], stg[96:128, 0:T])
                nc.sync.dma_start(t8b[64:96, 1, :], stg[96:128, T : 2 * T])
                nc.sync.dma_start(t8b[0:64, 1, :], stg[64:128, 2 * T : 3 * T])

            # ---------------- v projection (fp16, full precision) ----------
            with tc.tile_pool(name="vps", bufs=2, space="PSUM") as vps:
                for tb in range(KB):
                    ps = vps.tile([128, VC], f32, tag="vps", name="vps")
                    for kc in range(CB):
                        nc.tensor.matmul(
                            ps[:],
                            xft[:, kc, 128 * tb : 128 * (tb + 1)],
                            wvt[:, kc, :],
                            start=(kc == 0),
                            stop=(kc == CB - 1),
                        )
                    vre = vt[tb].rearrange("p (h c) -> p h c", c=D + 1)
                    pre = ps.rearrange("p (h c) -> p h c", c=D + 1)
                    nc.scalar.copy(vre[:, :, 0:D], pre[:, :, 0:D])
                    nc.gpsimd.memset(vre[:, :, D : D + 1], 1.0)

        # ---------------- attention ----------------
        pick_exp = _exp_engine_picker()
        with tc.tile_pool(name="stp", bufs=2, space="PSUM") as stp, tc.tile_pool(
            name="opp", bufs=2, space="PSUM"
        ) as opp:
            for half in range(2):
                lo = HT * half
                hi = lo + HT
                qb0 = lo // 128
                opre = None
                for h in range(HPC):
                    if h % 2 == 0:
                        opre = opre_p.tile([128, 8, 128], f16, tag="opre", name="opre")
                    if h < 3:
                        q8, k8, base = q8a, k8a, 32 * h
                    elif h == 3:
                        q8, k8, base = q8b, k8b, 64
                    else:
                        q8, k8, base = q8b, k8b, 32 * (h - 4)
                    bp = D * (h % 2)
                    opA = opp.tile([128, 7 * 65], f32, tag="opA", name="opA")
                    opB = opp.tile([128, 65], f32, tag="opB", name="opB")

                    def optile(j):
                        return opA[:, 65 * j : 65 * j + 65] if j < 7 else opB[:]

                    def emit_qk(kb):
                        # QK^T: S.T[k, q] fp8 DoubleRow, bank-split pieces
                        qs = 128 * kb
                        astart = max(lo, qs)
                        st = stp.tile([128, HT], f32, tag="st", name="st")
                        a = astart
                        while a < hi:
                            b = min(astart + 512 * ((a - astart) // 512 + 1), hi)
                            nc.tensor.matmul(
                                st[:, a - astart : b - astart],
                                k8[base : base + 32, :, qs : qs + 128],
                                q8[base : base + 32, :, a:b],
                                start=True,
                                stop=True,
                                perf_mode=DR,
                            )
                            a = b
                        return st

                    st = emit_qk(0)
                    for kb in range(hi // 128):
                        qs = 128 * kb
                        astart = max(lo, qs)
                        span = hi - astart
                        es = esp.tile([128, HT], f16, tag="es", name="es")
                        # exp (engine-balanced)
                        eng = pick_exp(span)
                        if eng == "act":
                            nc.scalar.activation(
                                es[:, :span], st[:, :span], Exp, scale=SCALE
                            )
                        else:
                            nc.vector.tensor_scalar(
                                es.bitcast(i16)[:, :span],
                                st[:, :span],
                                EXP_A,
                                EXP_B,
                                op0=MULT,
                                op1=ADD,
                            )
                        # software pipeline: next key block's QK overlaps exp
                        if kb + 1 < hi // 128:
                            st = emit_qk(kb + 1)
                        if qs >= lo:
                            # causal mask on the diagonal block (first 128 cols)
                            nc.gpsimd.affine_select(
                                out=es[:, 0:128],
                                in_=es[:, 0:128],
                                compare_op=mybir.AluOpType.is_ge,
                                fill=0.0,
                                base=0,
                                pattern=[[1, 128]],
                                channel_multiplier=-1,
                            )
                        # PV accumulate: op[qb] += es[:, qb].T @ V'
                        for qb in range(max(kb, qb0), hi // 128):
                            j = qb - qb0
                            nc.tensor.matmul(
                                optile(j),
                                es[:, 128 * qb - astart : 128 * qb - astart + 128],
                                vt[kb][:, 65 * h : 65 * h + 65],
                                start=(kb == 0 and j in (0, 7)),
                                stop=(kb == qb0 + 6 and j == 6)
                                or (kb == qb0 + 7 and j == 7),
                            )
                        # normalization as soon as each bank completes
                        if kb == qb0 + 6:
                            rA = nrm.tile([128, 8], f32, tag="rA", name="rA")
                            den = opA.rearrange("p (j c) -> p j c", c=65)
                            nc.vector.reciprocal_approx_fast(
                                out=rA[:, 0:7], in_=den[:, :, D : D + 1]
                            )
                            nc.vector.tensor_tensor(
                                opre[:, 0:7, bp : bp + D],
                                den[:, :, 0:D],
                                rA[:, 0:7, None].to_broadcast([128, 7, D]),
                                MULT,
                            )
                        if kb == qb0 + 7:
                            rB = nrm.tile([128, 1], f32, tag="rB", name="rB")
                            nc.vector.reciprocal_approx_fast(
                                out=rB[:], in_=opB[:, D : D + 1]
                            )
                            nc.scalar.activation(
                                opre[:, 7, bp : bp + D],
                                opB[:, 0:D],
                                Copy,
                                scale=rB[:, 0:1],
                            )
                    if h % 2 == 1:
                        # O^T for the head pair via batched DMA XBAR transpose
                        nc.sync.dma_start(
                            ocat[h // 2][:, lo:hi].rearrange(
                                "p (b q) -> p b q", q=128
                            ),
                            opre[:],
                            transpose=True,
                        )

        if dbg is not None:
            nc.sync.dma_start(dbg["q8a"][:], q8a[:])
            nc.sync.dma_start(dbg["k8a"][:], k8a[:])
            nc.sync.dma_start(dbg["q8b"][:], q8b[:])
            nc.sync.dma_start(dbg["k8b"][:], k8b[:])
            for t in range(KB):
                nc.sync.dma_start(dbg["v"][128 * t : 128 * (t + 1), :], vt[t][:])
            for i in range(3):
                nc.sync.dma_start(dbg["oc"][128 * i : 128 * (i + 1), :], ocat[i][:])

        # ---------------- output projection (f16) ----------------
        with tc.tile_pool(name="yps", bufs=2, space="PSUM") as yps:
            for tb in range(KB):
                ps = yps.tile([128, C], f32, tag="yps", name="yps")
                for kc in range(3):
                    for a, w in ((0, 512), (512, 256)):
                        nc.tensor.matmul(
                            ps[:, a : a + w],
                            ocat[kc][:, 128 * tb : 128 * (tb + 1)],
                            wpt[:, kc, a : a + w],
                            start=(kc == 0),
                            stop=(kc == 2),
                        )
                yt = yop.tile([128, C], f16, tag="yt", name="yt")
                if tb % 2 == 0:
                    nc.vector.tensor_copy(yt[:], ps[:])
                else:
                    nc.scalar.activation(yt[:], ps[:], Copy)
                nc.sync.dma_start(y[128 * tb : 128 * (tb + 1), :], yt[:])


_PROGRAM = None
_PROGRAM_DBG = None


def _build(debug=False):
    global _PROGRAM, _PROGRAM_DBG
    if not debug and _PROGRAM is not None:
        return _PROGRAM
    if debug and _PROGRAM_DBG is not None:
        return _PROGRAM_DBG
    nc = bacc.Bacc("TRN2", target_bir_lowering=False, debug=False, num_devices=NCORES)
    x8d = nc.dram_tensor("x8", [128, CB, T], f8, kind="ExternalInput").ap()
    w8d = nc.dram_tensor("w8", [128, CB, 768], f8, kind="ExternalInput").ap()
    bqd = nc.dram_tensor("bq", [128, 6], f32, kind="ExternalInput").ap()
    xfd = nc.dram_tensor("xf", [128, CB, T], f16, kind="ExternalInput").ap()
    wvd = nc.dram_tensor("wv", [128, CB, VC], f16, kind="ExternalInput").ap()
    wpd = nc.dram_tensor("wp", [128, 3, 768], f16, kind="ExternalInput").ap()
    y = nc.dram_tensor("y", [T, C], f16, kind="ExternalOutput").ap()
    dbg = None
    if debug:
        dbg = {
            "q8a": nc.dram_tensor("dbg_q8a", [128, 2, T], f8, kind="ExternalOutput").ap(),
            "k8a": nc.dram_tensor("dbg_k8a", [128, 2, T], f8, kind="ExternalOutput").ap(),
            "q8b": nc.dram_tensor("dbg_q8b", [128, 2, T], f8, kind="ExternalOutput").ap(),
            "k8b": nc.dram_tensor("dbg_k8b", [128, 2, T], f8, kind="ExternalOutput").ap(),
            "v": nc.dram_tensor("dbg_v", [KB * 128, VC], f16, kind="ExternalOutput").ap(),
            "oc": nc.dram_tensor("dbg_oc", [3 * 128, T], f16, kind="ExternalOutput").ap(),
        }
    with tile.TileContext(nc) as tc:
        _emit(tc, x8d, w8d, bqd, xfd, wvd, wpd, y, dbg)
    nc.compile()
    if debug:
        _PROGRAM_DBG = nc
    else:
        _PROGRAM = nc
    return nc


def _feat_index(c):
    """Map column c (0..127) of a 'g==2' proj block to (head_local, d)."""
    grp, r = divmod(c, 32)
    return (4 + grp % 2, r + 32 * (grp // 2))


def _in_maps(x, w_qkv, b_qkv, w_proj):
    maps = []
    for core in range(NCORES):
        b = core // 2
        half = core % 2
        h0 = HPC * half

        x8 = np.empty((128, CB, T), dtype=F8NP)
        xf = np.empty((128, CB, T), dtype=F16NP)
        for kc in range(CB):
            xT = x[b, :, 128 * kc : 128 * (kc + 1)].T
            x8[:, kc, :] = xT.astype(F8NP)
            xf[:, kc, :] = xT.astype(F16NP)

        # q/k weights + biases in DR column order
        w8 = np.empty((128, CB, 768), dtype=F8NP)
        bq = np.empty((128, 6), dtype=np.float32)
        for ob in range(6):
            base = 0 if ob < 3 else C
            g = ob % 3
            rows = np.empty(128, dtype=np.int64)
            for c in range(128):
                if g < 2:
                    hh, d = c // 32, 32 * g + c % 32
                else:
                    hh, d = _feat_index(c)
                rows[c] = base + D * (h0 + hh) + d
            wblk = w_qkv[rows, :] * WSCALE  # [128 feats, C]
            for kc in range(CB):
                w8[:, kc, 128 * ob : 128 * (ob + 1)] = (
                    wblk[:, 128 * kc : 128 * (kc + 1)].T.astype(F8NP)
                )
            bq[:, ob] = b_qkv[rows]

        wvm = np.zeros((128, CB, VC), dtype=F16NP)
        for hh in range(HPC):
            g = 2 * C + D * (h0 + hh)
            wv = w_qkv[g : g + D, :]  # [64, C]
            for kc in range(CB):
                wvm[:, kc, 65 * hh : 65 * hh + D] = (
                    wv[:, 128 * kc : 128 * (kc + 1)].T.astype(F16NP)
                )

        wp = np.empty((128, 3, 768), dtype=F16NP)
        r0 = D * h0
        for kc in range(3):
            wp[:, kc, :] = (
                w_proj[:, r0 + 128 * kc : r0 + 128 * (kc + 1)].T.astype(F16NP)
            )

        maps.append({"x8": x8, "w8": w8, "bq": bq, "xf": xf, "wv": wvm, "wp": wp})
    return maps


def kernel(x, w_qkv, b_qkv, w_proj, b_proj, _trace=False, _debug=False):
    x = np.asarray(x, dtype=np.float32)
    w_qkv = np.asarray(w_qkv, dtype=np.float32)
    b_qkv = np.asarray(b_qkv, dtype=np.float32)
    w_proj = np.asarray(w_proj, dtype=np.float32)
    b_proj = np.asarray(b_proj, dtype=np.float32)

    nc = _build(debug=_debug)
    maps = _in_maps(x, w_qkv, b_qkv, w_proj)
    res = bass_utils.run_bass_kernel_spmd(
        nc, maps, core_ids=list(range(NCORES)), trace=_trace
    )
    # constant bias term: b_proj + Wp_half @ bv_half summed over halves
    bv = b_qkv[2 * C :]
    bias = b_proj + w_proj @ bv
    out = np.empty((B, T, C), dtype=np.float32)
    for b in range(B):
        out[b] = (
            res.results[2 * b]["y"].astype(np.float32)
            + res.results[2 * b + 1]["y"].astype(np.float32)
            + bias
        )
    if _trace:
        kernel._last_exec_time_ns = res.exec_time_ns
    kernel._last_results = res
    return out
